# revision 1
# baseline (speedup 1.0000x reference)
"""Trainium2 Bass kernel for nn_MultiHeadedAttention_257698038597.

Multi-headed attention with channels: query/key/value [B=2,T=512,C=8,D=512],
mask [B,T,T,1]; four Linear(512,512) layers. Sharding: data-parallel over the
16 (b,c) pairs -> 2 units per core across 8 cores (per-core SPMD, no
collectives).

Per-core program, all matmul operands bf16 (fp32 PSUM accumulation):
  - host pre-transposes activations to x^T [din,T] and weights to chunked
    [128, 4, D] bf16; projections run as 16 accumulating matmuls each, with
    q/k bias applied during the PSUM eviction (unit 0 on the DVE, unit 1
    on the Scalar engine -- on the in-order DVE queue unit 1's evictions
    would sit behind unit 0's normalizer work and stall its scores) and the v bias folded into the
    host-side output bias (softmax rows sum to 1).
  - scores^T[s,t] per head with the K=64 head pairs packed onto partition
    halves 0-63/64-127 (disjoint PE row groups -> array concurrency on HW).
  - mask: the scores PSUM is preloaded with (mask-1)*240 via fp8-e4m3
    DoubleRow identity matmuls (0.5 cyc/col; {0,-240} are exact in e4m3,
    zero partner slots) so exp(0.125*psum) decays masked entries by e^-30;
    no elementwise mask pass needed anywhere.
  - softmax: exp on ScalarE (scale=1/8, bf16 out); normalizer = ones-column
    Z row of the att@v PSUM -> tensor_copy to partition 0 (plain copies may
    cross partitions; custom DVE ops may NOT), reciprocal_approx_fast in
    place, Pool broadcast, multiply during the PSUM eviction. The last
    group's chain is split along T to halve the tail latency.
  - y = att^T @ Wo evicted bf16 on ScalarE; the output bias (bv@Wo + bo) is
    added on the host in fp32 during assembly.
  - DMA: one descriptor-dense transfer per tensor (HWDGE costs ~0.6us of
    issuing-queue time each), split across the SP and Act queues in
    first-use order; wk/wo deferred behind the first unit's activation
    loads. Engine budget (CoreSim): PE 62.4us busy / DVE 47.2 / Act 47.2 /
    SP 15.9 / Pool 6.8, span 75.6us.
"""
import numpy as np

import concourse.bass as bass
import concourse.mybir as mybir
import concourse.tile as tile
from concourse import bacc
from concourse.bass import ts

P = 128
B, T, C, D = 2, 512, 8, 512
H, DK = 8, 64
KO = D // P             # 4 contraction chunks
U = 2                   # units (b,c pairs) per core
VS = 66                 # v_sb per-head stride: 64 v cols + 1 ones + 1 pad
NCORES = 8

F32 = mybir.dt.float32
BF16 = mybir.dt.bfloat16
F8E4 = mybir.dt.float8e4

EXP = mybir.ActivationFunctionType.Exp
IDENT = mybir.ActivationFunctionType.Identity
MUL = mybir.AluOpType.mult


def build_nc(repeat=1):
    nc = bacc.Bacc("TRN2", target_bir_lowering=False, debug=False)

    xqt = nc.dram_tensor("xqt", [U, P, KO, T], BF16, kind="ExternalInput")
    xkt = nc.dram_tensor("xkt", [U, P, KO, T], BF16, kind="ExternalInput")
    xvt = nc.dram_tensor("xvt", [U, P, KO, T], BF16, kind="ExternalInput")
    # additive mask bias (mask-1)*240 transposed [s, t], fp8 e4m3 (exact for
    # {0, -240}) in DoubleRow pair layout with zero partners; identity
    # likewise. DoubleRow halves the PE cost of the mask preloads.
    mbias = nc.dram_tensor("mbias", [P, KO, 2, T], F8E4, kind="ExternalInput")
    iden = nc.dram_tensor("iden", [P, 2, P], F8E4, kind="ExternalInput")
    wq = nc.dram_tensor("wq", [P, KO, D], BF16, kind="ExternalInput")
    wk = nc.dram_tensor("wk", [P, KO, D], BF16, kind="ExternalInput")
    wv = nc.dram_tensor("wv", [P, KO, D], BF16, kind="ExternalInput")
    wo = nc.dram_tensor("wo", [P, KO, D], BF16, kind="ExternalInput")
    bqd = nc.dram_tensor("bqd", [P, KO], F32, kind="ExternalInput")
    bkd = nc.dram_tensor("bkd", [P, KO], F32, kind="ExternalInput")
    y = nc.dram_tensor("y", [U, KO, P, D], BF16, kind="ExternalOutput")

    with tile.TileContext(nc) as tc:
        import contextlib
        with contextlib.ExitStack() as ctx:
            const = ctx.enter_context(tc.tile_pool(name="const", bufs=1))
            xt_pool = ctx.enter_context(tc.tile_pool(name="xt", bufs=3))
            qk_pool = ctx.enter_context(tc.tile_pool(name="qk", bufs=3))
            p_pool = ctx.enter_context(tc.tile_pool(name="pp", bufs=6))
            att_pool = ctx.enter_context(tc.tile_pool(name="att", bufs=2))
            nrm_pool = ctx.enter_context(tc.tile_pool(name="nrm", bufs=3))
            y_pool = ctx.enter_context(tc.tile_pool(name="y", bufs=3))
            ps_proj = ctx.enter_context(tc.tile_pool(name="psp", bufs=2, space="PSUM"))
            ps_sc = ctx.enter_context(tc.tile_pool(name="pssc", bufs=2, space="PSUM"))
            ps_av = ctx.enter_context(tc.tile_pool(name="psav", bufs=2, space="PSUM"))

            # constants: weights + mask + biases on the scalar queue
            # (front-loaded before the exp work begins)
            wq_sb = const.tile([P, KO, D], BF16, tag="wq")
            wk_sb = const.tile([P, KO, D], BF16, tag="wk")
            wv_sb = const.tile([P, KO, D], BF16, tag="wv")
            wo_sb = const.tile([P, KO, D], BF16, tag="wo")
            mb_sb = const.tile([P, KO, 2, T], F8E4, tag="mb")
            id_sb = const.tile([P, 2, P], F8E4, tag="iden")
            bq_sb = const.tile([P, KO], F32, tag="bq")
            bk_sb = const.tile([P, KO], F32, tag="bk")
            # wq chunked per-ko so the first projection starts early; the
            # k/v operands follow in first-use order, split across the two
            # HWDGE queues (SP and Act)
            for ko in range(KO):
                nc.sync.dma_start(out=wq_sb[:, ko, :], in_=wq[:, ko, :])
            nc.scalar.dma_start(out=bq_sb, in_=bqd[:, :])
            nc.scalar.dma_start(out=bk_sb, in_=bkd[:, :])

            for _rep in range(repeat):
                att_units = []
                for u in range(U):
                    # ---- load transposed activations (SP queue, one DMA each)
                    xq_sb = xt_pool.tile([P, KO, T], BF16, tag="xq")
                    xk_sb = xt_pool.tile([P, KO, T], BF16, tag="xk")
                    xv_sb = xt_pool.tile([P, KO, T], BF16, tag="xv")
                    if u == 0:
                        for ko in range(KO):
                            # chunks split across both HWDGE queues so the
                            # projection inputs land in parallel
                            q_eng = nc.sync if ko < 2 else nc.scalar
                            q_eng.dma_start(out=xq_sb[:, ko, :],
                                            in_=xqt[u, :, ko, :])
                        nc.scalar.dma_start(out=xk_sb, in_=xkt[u, :, :, :])
                        nc.sync.dma_start(out=xv_sb, in_=xvt[u, :, :, :])
                        if _rep == 0:
                            # wk/wo/wv/id/mb deferred behind the first unit's
                            # x loads (first used at ~10-35us; the x chunks
                            # gate the first projections at ~6-9us)
                            nc.sync.dma_start(out=wk_sb, in_=wk[:, :, :])
                            nc.scalar.dma_start(out=wv_sb, in_=wv[:, :, :])
                            nc.scalar.dma_start(out=id_sb, in_=iden[:, :, :])
                            nc.scalar.dma_start(out=mb_sb,
                                                in_=mbias[:, :, :, :])
                            nc.sync.dma_start(out=wo_sb, in_=wo[:, :, :])
                    else:
                        nc.sync.dma_start(out=xq_sb, in_=xqt[u, :, :, :])
                        nc.scalar.dma_start(out=xk_sb, in_=xkt[u, :, :, :])
                        nc.sync.dma_start(out=xv_sb, in_=xvt[u, :, :, :])

                    # ---- projections
                    qT_sb = qk_pool.tile([P, KO, T], BF16, tag="qT")
                    kT_sb = qk_pool.tile([P, KO, T], BF16, tag="kT")
                    v_sb = qk_pool.tile([P, KO, H * VS], BF16, tag="v")
                    # ones columns (position 64 of each 66-wide head block)
                    ones_view = bass.AP(
                        tensor=v_sb.tensor, offset=v_sb[:, 0, 64].offset,
                        ap=[list(v_sb[:].ap[0]), [H * VS, KO], [VS, H], [1, 2]])
                    nc.vector.memset(ones_view, 1.0)

                    for mo in range(KO):
                        psq = ps_proj.tile([P, T], F32, tag="psp")
                        for ko in range(KO):
                            nc.tensor.matmul(psq[:], wq_sb[:, ko, ts(mo, P)],
                                             xq_sb[:, ko, :],
                                             start=(ko == 0), stop=(ko == KO - 1))
                        if u == 0:
                            nc.vector.tensor_scalar_add(
                                qT_sb[:, mo, :], psq[:], bq_sb[:, mo, None])
                        else:
                            # unit 1's evictions go to the Act queue: on DVE
                            # they'd sit behind unit 0's normalizer FIFO and
                            # stall unit 1's first scores by ~1us
                            nc.scalar.activation(qT_sb[:, mo, :], psq[:],
                                                 IDENT, bias=bq_sb[:, mo, None])
                    for mo in range(KO):
                        psk = ps_proj.tile([P, T], F32, tag="psp")
                        for ko in range(KO):
                            nc.tensor.matmul(psk[:], wk_sb[:, ko, ts(mo, P)],
                                             xk_sb[:, ko, :],
                                             start=(ko == 0), stop=(ko == KO - 1))
                        if u == 0:
                            nc.vector.tensor_scalar_add(
                                kT_sb[:, mo, :], psk[:], bk_sb[:, mo, None])
                        else:
                            nc.scalar.activation(kT_sb[:, mo, :], psk[:],
                                                 IDENT, bias=bk_sb[:, mo, None])
                    for mo in range(KO):
                        psv = ps_proj.tile([P, T], F32, tag="psp")
                        for ko in range(KO):
                            nc.tensor.matmul(psv[:], xv_sb[:, ko, ts(mo, P)],
                                             wv_sb[:, ko, :],
                                             start=(ko == 0), stop=(ko == KO - 1))
                        # scatter into per-head 66-strided blocks (no bias: bv
                        # is folded into the host-side output bias)
                        v_dst = bass.AP(
                            tensor=v_sb.tensor, offset=v_sb[:, mo, 0].offset,
                            ap=[list(v_sb[:].ap[0]), [VS, H], [1, DK]])
                        nc.vector.tensor_copy(out=v_dst,
                                              in_=psv[:].rearrange("p (h d) -> p h d", h=H))

                    # ---- attention, head pairs (heads 2g / 2g+1 live on
                    # partition halves 0-63 / 64-127 of kT/qT chunk g)
                    att_k = [att_pool.tile([P, T], BF16, tag=f"attT{ko}",
                                           name=f"attT{ko}_u{u}")
                             for ko in range(KO)]
                    att_units.append(att_k)
                    for g in range(4):
                        heads = (2 * g, 2 * g + 1)
                        p_tiles = []
                        for so in range(KO):
                            sc = ps_sc.tile([P, 2, T], F32, tag="sc")
                            # preload the additive mask bias via fp8 DoubleRow
                            # identity matmuls (0.5 cyc/col), then the two
                            # K=64 scores matmuls
                            for j in range(2):
                                nc.tensor.matmul(
                                    sc[:, j, :], id_sb[:, :, :],
                                    mb_sb[:, so, :, :],
                                    start=True, stop=False,
                                    perf_mode=mybir.MatmulPerfMode.DoubleRow)
                            for j, h in enumerate(heads):
                                lo = 64 * (h % 2)
                                nc.tensor.matmul(
                                    sc[:, j, :],
                                    kT_sb[lo:lo + 64, g, ts(so, P)],
                                    qT_sb[lo:lo + 64, g, :],
                                    start=False, stop=True)
                            pt = p_pool.tile([P, 2, T], BF16, tag="p")
                            nc.scalar.activation(pt[:], sc[:], EXP, scale=0.125)
                            p_tiles.append(pt)

                        avs = []
                        for j, h in enumerate(heads):
                            av = ps_av.tile([P, T], F32, tag="av")
                            for so in range(KO):
                                nc.tensor.matmul(
                                    av[0:65, :],
                                    v_sb[:, so, VS * h:VS * h + 65],
                                    p_tiles[so][:, j, :],
                                    start=(so == 0), stop=(so == KO - 1))
                            avs.append(av)
                        # normalizer: Z row copy to partition 0 (plain
                        # tensor_copy handles the partition crossing; the
                        # custom reciprocal op cannot), reciprocal in place,
                        # Pool broadcast, multiply during PSUM eviction.
                        # The last group of the last unit gates the final
                        # output projections, so its chain is split along T
                        # to halve the tail latency.
                        nsplit = 2 if (u == U - 1 and g == 3) else 1
                        TS2 = T // nsplit
                        for j, h in enumerate(heads):
                            zz = nrm_pool.tile([1, T], F32, tag="zz")
                            bc = nrm_pool.tile([64, T], F32, tag="bc")
                            lo = 64 * (h % 2)
                            for piece in range(nsplit):
                                sl = slice(piece * TS2, (piece + 1) * TS2)
                                nc.vector.tensor_copy(out=zz[0:1, sl],
                                                      in_=avs[j][64:65, sl])
                                nc.vector.reciprocal_approx_fast(
                                    out=zz[0:1, sl], in_=zz[0:1, sl])
                                nc.gpsimd.partition_broadcast(bc[:, sl],
                                                              zz[0:1, sl])
                                nc.vector.tensor_tensor(
                                    out=att_k[g][lo:lo + 64, sl],
                                    in0=avs[j][0:64, sl], in1=bc[:, sl],
                                    op=MUL)

                # ---- output projections, emitted after both units so they
                # can fill PE gaps; bf16 eviction, bias added host-side
                for u in range(U):
                    att_k = att_units[u]
                    # two t-chunks interleaved by ko: their ko<3 matmuls all
                    # sit AHEAD of the att_k[3]-gated ko=3 matmuls in the
                    # in-order PE queue, doubling the work that can run while
                    # the last normalizer chain completes
                    for tcp in range(0, KO, 2):
                        psys = [ps_proj.tile([P, T], F32, tag="psp",
                                             name=f"psy_u{u}t{tcp + d}")
                                for d in range(2)]
                        for ko in range(KO):
                            for d in range(2):
                                nc.tensor.matmul(
                                    psys[d][:],
                                    att_k[ko][:, ts(tcp + d, P)],
                                    wo_sb[:, ko, :],
                                    start=(ko == 0), stop=(ko == KO - 1))
                        for d in range(2):
                            tc_i = tcp + d
                            y_sb = y_pool.tile([P, D], BF16, tag="y")
                            nc.scalar.copy(out=y_sb[:], in_=psys[d][:])
                            nc.sync.dma_start(out=y[u, tc_i, :, :],
                                              in_=y_sb[:])

    nc.compile()
    return nc


_NC_CACHE = {}


def _get_nc(repeat=1):
    if repeat not in _NC_CACHE:
        _NC_CACHE[repeat] = build_nc(repeat)
    return _NC_CACHE[repeat]


def _chunkT(x):
    """[T,D] -> x^T chunked [P, KO, T] (din = ko*128 + p)."""
    return np.ascontiguousarray(x.T.reshape(KO, P, T).transpose(1, 0, 2))


def _chunkW(w):
    """[D,D] (in,out) -> [P, KO, D]."""
    return np.ascontiguousarray(w.reshape(KO, P, D).transpose(1, 0, 2))


def make_in_maps(query, key, value, mask, Wq, bq, Wk, bk, Wv, bv, Wo, bo):
    import ml_dtypes
    bf16 = ml_dtypes.bfloat16
    f8 = ml_dtypes.float8_e4m3
    query = np.asarray(query, np.float32)
    key = np.asarray(key, np.float32)
    value = np.asarray(value, np.float32)
    mask = np.asarray(mask)
    Wq, Wk, Wv, Wo = (np.asarray(w, np.float32) for w in (Wq, Wk, Wv, Wo))
    bq, bk, bv, bo = (np.asarray(b, np.float32) for b in (bq, bk, bv, bo))

    wq_h = _chunkW(Wq).astype(bf16)
    wk_h = _chunkW(Wk).astype(bf16)
    wv_h = _chunkW(Wv).astype(bf16)
    wo_h = _chunkW(Wo).astype(bf16)
    bq_h = np.ascontiguousarray(bq.reshape(KO, P).T)
    bk_h = np.ascontiguousarray(bk.reshape(KO, P).T)

    in_maps = []
    for core in range(NCORES):
        b = core // 4
        cs = [2 * (core % 4), 2 * (core % 4) + 1]
        xq = np.stack([_chunkT(query[b, :, c, :]) for c in cs]).astype(bf16)
        xk = np.stack([_chunkT(key[b, :, c, :]) for c in cs]).astype(bf16)
        xv = np.stack([_chunkT(value[b, :, c, :]) for c in cs]).astype(bf16)
        # mask bias (mask^T - 1) * 240 in DoubleRow pair layout [P,KO,2,T]
        # (slot 1 zeroed so the zero partner weights contribute nothing)
        mt = _chunkT(mask[b, :, :, 0].astype(np.float32))
        mb2 = np.zeros((P, KO, 2, T), np.float32)
        mb2[:, :, 0, :] = (mt - 1.0) * 240.0
        id2 = np.zeros((P, 2, P), np.float32)
        id2[:, 0, :] = np.eye(P, dtype=np.float32)
        in_maps.append({
            "xqt": xq, "xkt": xk, "xvt": xv,
            "mbias": mb2.astype(f8),
            "iden": id2.astype(f8),
            "wq": wq_h, "wk": wk_h, "wv": wv_h, "wo": wo_h,
            "bqd": bq_h, "bkd": bk_h,
        })
    return in_maps


def _out_bias(Wo, bv, bo):
    return (np.asarray(bv, np.float32) @ np.asarray(Wo, np.float32)
            + np.asarray(bo, np.float32))


def assemble(results, bo2):
    out = np.empty((B, T, C, D), np.float32)
    for core, res in enumerate(results):
        b = core // 4
        cs = [2 * (core % 4), 2 * (core % 4) + 1]
        yv = np.asarray(res["y"], np.float32)  # [U, KO, P, D]
        for u, c in enumerate(cs):
            out[b, :, c, :] = yv[u].reshape(T, D) + bo2[None, :]
    return out


def kernel(**inputs):
    from concourse.bass_utils import run_bass_kernel_spmd
    nc = _get_nc()
    in_maps = make_in_maps(**inputs)
    res = run_bass_kernel_spmd(nc, in_maps, core_ids=list(range(NCORES)))
    bo2 = _out_bias(inputs["Wo"], inputs["bv"], inputs["bo"])
    return assemble(res.results, bo2)



# revision 10
# speedup vs baseline: 1.3510x; 1.3510x over previous
"""Trainium2 Bass kernel for nn_MultiHeadedAttention_257698038597.

Multi-headed attention with channels: query/key/value [B=2,T=512,C=8,D=512],
mask [B,T,T,1]; four Linear(512,512) layers. Sharding: data-parallel over the
16 (b,c) pairs -> 2 units per core across 8 cores (per-core SPMD, no
collectives).

Per-core program, all matmul operands bf16 (fp32 PSUM accumulation):
  - host pre-transposes activations to x^T [din,T] and weights to chunked
    [128, 4, D] bf16; projections run as 16 accumulating matmuls each, with
    q/k bias applied during the PSUM eviction (unit 0 on the DVE, unit 1
    on the Scalar engine -- on the in-order DVE queue unit 1's evictions
    would sit behind unit 0's normalizer work and stall its scores) and the v bias folded into the
    host-side output bias (softmax rows sum to 1).
  - scores^T[s,t] per head with the K=64 head pairs packed onto partition
    halves 0-63/64-127 (disjoint PE row groups -> array concurrency on HW).
  - mask: the scores PSUM is preloaded with (mask-1)*240 via fp8-e4m3
    DoubleRow identity matmuls (0.5 cyc/col; {0,-240} are exact in e4m3,
    zero partner slots) so exp(0.125*psum) decays masked entries by e^-30;
    no elementwise mask pass needed anywhere.
  - softmax: exp on ScalarE (scale=1/8, bf16 out); normalizer = ones-column
    Z row of the att@v PSUM -> tensor_copy to partition 0 (plain copies may
    cross partitions; custom DVE ops may NOT), reciprocal_approx_fast in
    place, Pool broadcast, multiply during the PSUM eviction. The last
    group's chain is split along T to halve the tail latency.
  - y = att^T @ Wo evicted bf16 on ScalarE; the output bias (bv@Wo + bo) is
    added on the host in fp32 during assembly.
  - DMA: one descriptor-dense transfer per tensor (HWDGE costs ~0.6us of
    issuing-queue time each), split across the SP and Act queues in
    first-use order; wk/wo deferred behind the first unit's activation
    loads. Engine budget (CoreSim): PE 62.4us busy / DVE 47.2 / Act 47.2 /
    SP 15.9 / Pool 6.8, span 75.6us.
"""
import os

import numpy as np

import concourse.bass as bass
import concourse.mybir as mybir
import concourse.tile as tile
from concourse import bacc
from concourse.bass import ts

P = 128
B, T, C, D = 2, 512, 8, 512
H, DK = 8, 64
KO = D // P             # 4 contraction chunks
U = 2                   # units (b,c pairs) per core
VS = 66                 # v_sb per-head stride: 64 v cols + 1 ones + 1 pad
NCORES = 8

F32 = mybir.dt.float32
BF16 = mybir.dt.bfloat16
F8E4 = mybir.dt.float8e4

EXP = mybir.ActivationFunctionType.Exp
IDENT = mybir.ActivationFunctionType.Identity
MUL = mybir.AluOpType.mult


def build_nc(repeat=1):
    nc = bacc.Bacc("TRN2", target_bir_lowering=False, debug=False)

    xqt = nc.dram_tensor("xqt", [U, P, KO, T], BF16, kind="ExternalInput")
    xkt = nc.dram_tensor("xkt", [U, P, KO, T], BF16, kind="ExternalInput")
    xvt = nc.dram_tensor("xvt", [U, P, KO, T], BF16, kind="ExternalInput")
    # additive mask bias (mask-1)*240 transposed [s, t], fp8 e4m3 (exact for
    # {0, -240}) in DoubleRow pair layout with zero partners; identity
    # likewise. DoubleRow halves the PE cost of the mask preloads.
    mbias = nc.dram_tensor("mbias", [P, KO, 2, T], F8E4, kind="ExternalInput")
    iden = nc.dram_tensor("iden", [P, 2, P], F8E4, kind="ExternalInput")
    wq = nc.dram_tensor("wq", [P, KO, D], BF16, kind="ExternalInput")
    wk = nc.dram_tensor("wk", [P, KO, D], BF16, kind="ExternalInput")
    wv = nc.dram_tensor("wv", [P, KO, D], BF16, kind="ExternalInput")
    wo = nc.dram_tensor("wo", [P, KO, D], BF16, kind="ExternalInput")
    bqd = nc.dram_tensor("bqd", [P, KO], F32, kind="ExternalInput")
    bkd = nc.dram_tensor("bkd", [P, KO], F32, kind="ExternalInput")
    y = nc.dram_tensor("y", [U, KO, P, D], BF16, kind="ExternalOutput")

    with tile.TileContext(nc) as tc:
        import contextlib
        with contextlib.ExitStack() as ctx:
            const = ctx.enter_context(tc.tile_pool(name="const", bufs=1))
            xt_pool = ctx.enter_context(tc.tile_pool(name="xt", bufs=3))
            qk_pool = ctx.enter_context(tc.tile_pool(name="qk", bufs=3))
            p_pool = ctx.enter_context(tc.tile_pool(name="pp", bufs=6))
            att_pool = ctx.enter_context(tc.tile_pool(name="att", bufs=2))
            nrm_pool = ctx.enter_context(tc.tile_pool(name="nrm", bufs=3))
            y_pool = ctx.enter_context(tc.tile_pool(name="y", bufs=3))
            ps_proj = ctx.enter_context(tc.tile_pool(name="psp", bufs=2, space="PSUM"))
            ps_sc = ctx.enter_context(tc.tile_pool(name="pssc", bufs=2, space="PSUM"))
            ps_av = ctx.enter_context(tc.tile_pool(name="psav", bufs=2, space="PSUM"))

            # constants: weights + mask + biases on the scalar queue
            # (front-loaded before the exp work begins)
            wq_sb = const.tile([P, KO, D], BF16, tag="wq")
            wk_sb = const.tile([P, KO, D], BF16, tag="wk")
            wv_sb = const.tile([P, KO, D], BF16, tag="wv")
            wo_sb = const.tile([P, KO, D], BF16, tag="wo")
            mb_sb = const.tile([P, KO, 2, T], F8E4, tag="mb")
            id_sb = const.tile([P, 2, P], F8E4, tag="iden")
            bq_sb = const.tile([P, KO], F32, tag="bq")
            bk_sb = const.tile([P, KO], F32, tag="bk")
            # wq chunked per-ko so the first projection starts early; the
            # k/v operands follow in first-use order, split across the two
            # HWDGE queues (SP and Act)
            for ko in range(KO):
                nc.sync.dma_start(out=wq_sb[:, ko, :], in_=wq[:, ko, :])
            nc.scalar.dma_start(out=bq_sb, in_=bqd[:, :])
            nc.scalar.dma_start(out=bk_sb, in_=bkd[:, :])

            for _rep in range(repeat):
                att_units = []
                for u in range(U):
                    # ---- load transposed activations (SP queue, one DMA each)
                    xq_sb = xt_pool.tile([P, KO, T], BF16, tag="xq")
                    xk_sb = xt_pool.tile([P, KO, T], BF16, tag="xk")
                    xv_sb = xt_pool.tile([P, KO, T], BF16, tag="xv")
                    if u == 0:
                        for ko in range(KO):
                            # chunks split across both HWDGE queues so the
                            # projection inputs land in parallel
                            q_eng = nc.sync if ko < 2 else nc.scalar
                            q_eng.dma_start(out=xq_sb[:, ko, :],
                                            in_=xqt[u, :, ko, :])
                        nc.scalar.dma_start(out=xk_sb, in_=xkt[u, :, :, :])
                        nc.sync.dma_start(out=xv_sb, in_=xvt[u, :, :, :])
                        if _rep == 0:
                            # wk/wo/wv/id/mb deferred behind the first unit's
                            # x loads (first used at ~10-35us; the x chunks
                            # gate the first projections at ~6-9us)
                            nc.sync.dma_start(out=wk_sb, in_=wk[:, :, :])
                            nc.scalar.dma_start(out=wv_sb, in_=wv[:, :, :])
                            nc.scalar.dma_start(out=id_sb, in_=iden[:, :, :])
                            nc.scalar.dma_start(out=mb_sb,
                                                in_=mbias[:, :, :, :])
                            nc.sync.dma_start(out=wo_sb, in_=wo[:, :, :])
                    else:
                        nc.sync.dma_start(out=xq_sb, in_=xqt[u, :, :, :])
                        nc.scalar.dma_start(out=xk_sb, in_=xkt[u, :, :, :])
                        nc.sync.dma_start(out=xv_sb, in_=xvt[u, :, :, :])

                    # ---- projections
                    qT_sb = qk_pool.tile([P, KO, T], BF16, tag="qT")
                    kT_sb = qk_pool.tile([P, KO, T], BF16, tag="kT")
                    v_sb = qk_pool.tile([P, KO, H * VS], BF16, tag="v")
                    # ones columns (position 64 of each 66-wide head block)
                    ones_view = bass.AP(
                        tensor=v_sb.tensor, offset=v_sb[:, 0, 64].offset,
                        ap=[list(v_sb[:].ap[0]), [H * VS, KO], [VS, H], [1, 2]])
                    nc.vector.memset(ones_view, 1.0)

                    for mo in range(KO):
                        psq = ps_proj.tile([P, T], F32, tag="psp")
                        for ko in range(KO):
                            nc.tensor.matmul(psq[:], wq_sb[:, ko, ts(mo, P)],
                                             xq_sb[:, ko, :],
                                             start=(ko == 0), stop=(ko == KO - 1))
                        if u == 0:
                            nc.vector.tensor_scalar_add(
                                qT_sb[:, mo, :], psq[:], bq_sb[:, mo, None])
                        else:
                            # unit 1's evictions go to the Act queue: on DVE
                            # they'd sit behind unit 0's normalizer FIFO and
                            # stall unit 1's first scores by ~1us
                            nc.scalar.activation(qT_sb[:, mo, :], psq[:],
                                                 IDENT, bias=bq_sb[:, mo, None])
                    for mo in range(KO):
                        psk = ps_proj.tile([P, T], F32, tag="psp")
                        for ko in range(KO):
                            nc.tensor.matmul(psk[:], wk_sb[:, ko, ts(mo, P)],
                                             xk_sb[:, ko, :],
                                             start=(ko == 0), stop=(ko == KO - 1))
                        if u == 0:
                            nc.vector.tensor_scalar_add(
                                kT_sb[:, mo, :], psk[:], bk_sb[:, mo, None])
                        else:
                            nc.scalar.activation(kT_sb[:, mo, :], psk[:],
                                                 IDENT, bias=bk_sb[:, mo, None])
                    for mo in range(KO):
                        psv = ps_proj.tile([P, T], F32, tag="psp")
                        for ko in range(KO):
                            nc.tensor.matmul(psv[:], xv_sb[:, ko, ts(mo, P)],
                                             wv_sb[:, ko, :],
                                             start=(ko == 0), stop=(ko == KO - 1))
                        # scatter into per-head 66-strided blocks (no bias: bv
                        # is folded into the host-side output bias)
                        v_dst = bass.AP(
                            tensor=v_sb.tensor, offset=v_sb[:, mo, 0].offset,
                            ap=[list(v_sb[:].ap[0]), [VS, H], [1, DK]])
                        nc.vector.tensor_copy(out=v_dst,
                                              in_=psv[:].rearrange("p (h d) -> p h d", h=H))

                    # ---- attention, head pairs (heads 2g / 2g+1 live on
                    # partition halves 0-63 / 64-127 of kT/qT chunk g)
                    att_k = [att_pool.tile([P, T], BF16, tag=f"attT{ko}",
                                           name=f"attT{ko}_u{u}")
                             for ko in range(KO)]
                    att_units.append(att_k)
                    for g in range(4):
                        heads = (2 * g, 2 * g + 1)
                        p_tiles = []
                        for so in range(KO):
                            sc = ps_sc.tile([P, 2, T], F32, tag="sc")
                            # preload the additive mask bias via fp8 DoubleRow
                            # identity matmuls (0.5 cyc/col), then the two
                            # K=64 scores matmuls
                            probe = os.environ.get("MASK_PROBE") == "1"
                            if not probe:
                                for j in range(2):
                                    nc.tensor.matmul(
                                        sc[:, j, :], id_sb[:, :, :],
                                        mb_sb[:, so, :, :],
                                        start=True, stop=False,
                                        perf_mode=mybir.MatmulPerfMode.DoubleRow)
                            for j, h in enumerate(heads):
                                lo = 64 * (h % 2)
                                nc.tensor.matmul(
                                    sc[:, j, :],
                                    kT_sb[lo:lo + 64, g, ts(so, P)],
                                    qT_sb[lo:lo + 64, g, :],
                                    start=probe, stop=True)
                            pt = p_pool.tile([P, 2, T], BF16, tag="p")
                            nc.scalar.activation(pt[:], sc[:], EXP, scale=0.125)
                            p_tiles.append(pt)

                        avs = []
                        for j, h in enumerate(heads):
                            av = ps_av.tile([P, T], F32, tag="av")
                            for so in range(KO):
                                nc.tensor.matmul(
                                    av[0:65, :],
                                    v_sb[:, so, VS * h:VS * h + 65],
                                    p_tiles[so][:, j, :],
                                    start=(so == 0), stop=(so == KO - 1))
                            avs.append(av)
                        # normalizer: Z row copy to partition 0 (plain
                        # tensor_copy handles the partition crossing; the
                        # custom reciprocal op cannot), reciprocal in place,
                        # Pool broadcast, multiply during PSUM eviction.
                        # The last group of the last unit gates the final
                        # output projections, so its chain is split along T
                        # to halve the tail latency.
                        nsplit = 2 if (u == U - 1 and g == 3) else 1
                        TS2 = T // nsplit
                        for j, h in enumerate(heads):
                            zz = nrm_pool.tile([1, T], F32, tag="zz")
                            bc = nrm_pool.tile([64, T], F32, tag="bc")
                            lo = 64 * (h % 2)
                            for piece in range(nsplit):
                                sl = slice(piece * TS2, (piece + 1) * TS2)
                                nc.vector.tensor_copy(out=zz[0:1, sl],
                                                      in_=avs[j][64:65, sl])
                                nc.vector.reciprocal_approx_fast(
                                    out=zz[0:1, sl], in_=zz[0:1, sl])
                                nc.gpsimd.partition_broadcast(bc[:, sl],
                                                              zz[0:1, sl])
                                nc.vector.tensor_tensor(
                                    out=att_k[g][lo:lo + 64, sl],
                                    in0=avs[j][0:64, sl], in1=bc[:, sl],
                                    op=MUL)

                # ---- output projections, emitted after both units so they
                # can fill PE gaps; bf16 eviction, bias added host-side
                for u in range(U):
                    att_k = att_units[u]
                    # two t-chunks interleaved by ko: their ko<3 matmuls all
                    # sit AHEAD of the att_k[3]-gated ko=3 matmuls in the
                    # in-order PE queue, doubling the work that can run while
                    # the last normalizer chain completes
                    for tcp in range(0, KO, 2):
                        psys = [ps_proj.tile([P, T], F32, tag="psp",
                                             name=f"psy_u{u}t{tcp + d}")
                                for d in range(2)]
                        for ko in range(KO):
                            for d in range(2):
                                nc.tensor.matmul(
                                    psys[d][:],
                                    att_k[ko][:, ts(tcp + d, P)],
                                    wo_sb[:, ko, :],
                                    start=(ko == 0), stop=(ko == KO - 1))
                        for d in range(2):
                            tc_i = tcp + d
                            y_sb = y_pool.tile([P, D], BF16, tag="y")
                            nc.scalar.copy(out=y_sb[:], in_=psys[d][:])
                            nc.sync.dma_start(out=y[u, tc_i, :, :],
                                              in_=y_sb[:])

    nc.compile()
    return nc


_NC_CACHE = {}


def _get_nc(repeat=1):
    if repeat not in _NC_CACHE:
        _NC_CACHE[repeat] = build_nc(repeat)
    return _NC_CACHE[repeat]


def _chunkT(x):
    """[T,D] -> x^T chunked [P, KO, T] (din = ko*128 + p)."""
    return np.ascontiguousarray(x.T.reshape(KO, P, T).transpose(1, 0, 2))


def _chunkW(w):
    """[D,D] (in,out) -> [P, KO, D]."""
    return np.ascontiguousarray(w.reshape(KO, P, D).transpose(1, 0, 2))


def make_in_maps(query, key, value, mask, Wq, bq, Wk, bk, Wv, bv, Wo, bo):
    import ml_dtypes
    bf16 = ml_dtypes.bfloat16
    f8 = ml_dtypes.float8_e4m3
    query = np.asarray(query, np.float32)
    key = np.asarray(key, np.float32)
    value = np.asarray(value, np.float32)
    mask = np.asarray(mask)
    Wq, Wk, Wv, Wo = (np.asarray(w, np.float32) for w in (Wq, Wk, Wv, Wo))
    bq, bk, bv, bo = (np.asarray(b, np.float32) for b in (bq, bk, bv, bo))

    wq_h = _chunkW(Wq).astype(bf16)
    wk_h = _chunkW(Wk).astype(bf16)
    wv_h = _chunkW(Wv).astype(bf16)
    wo_h = _chunkW(Wo).astype(bf16)
    bq_h = np.ascontiguousarray(bq.reshape(KO, P).T)
    bk_h = np.ascontiguousarray(bk.reshape(KO, P).T)

    in_maps = []
    for core in range(NCORES):
        b = core // 4
        cs = [2 * (core % 4), 2 * (core % 4) + 1]
        xq = np.stack([_chunkT(query[b, :, c, :]) for c in cs]).astype(bf16)
        xk = np.stack([_chunkT(key[b, :, c, :]) for c in cs]).astype(bf16)
        xv = np.stack([_chunkT(value[b, :, c, :]) for c in cs]).astype(bf16)
        # mask bias (mask^T - 1) * 240 in DoubleRow pair layout [P,KO,2,T]
        # (slot 1 zeroed so the zero partner weights contribute nothing)
        mt = _chunkT(mask[b, :, :, 0].astype(np.float32))
        mb2 = np.zeros((P, KO, 2, T), np.float32)
        mb2[:, :, 0, :] = (mt - 1.0) * 240.0
        id2 = np.zeros((P, 2, P), np.float32)
        id2[:, 0, :] = np.eye(P, dtype=np.float32)
        in_maps.append({
            "xqt": xq, "xkt": xk, "xvt": xv,
            "mbias": mb2.astype(f8),
            "iden": id2.astype(f8),
            "wq": wq_h, "wk": wk_h, "wv": wv_h, "wo": wo_h,
            "bqd": bq_h, "bkd": bk_h,
        })
    return in_maps


def _out_bias(Wo, bv, bo):
    return (np.asarray(bv, np.float32) @ np.asarray(Wo, np.float32)
            + np.asarray(bo, np.float32))


def assemble(results, bo2):
    out = np.empty((B, T, C, D), np.float32)
    for core, res in enumerate(results):
        b = core // 4
        cs = [2 * (core % 4), 2 * (core % 4) + 1]
        yv = np.asarray(res["y"], np.float32)  # [U, KO, P, D]
        for u, c in enumerate(cs):
            out[b, :, c, :] = yv[u].reshape(T, D) + bo2[None, :]
    return out


def kernel(**inputs):
    from concourse.bass_utils import run_bass_kernel_spmd
    nc = _get_nc()
    in_maps = make_in_maps(**inputs)
    res = run_bass_kernel_spmd(nc, in_maps, core_ids=list(range(NCORES)))
    bo2 = _out_bias(inputs["Wo"], inputs["bv"], inputs["bo"])
    return assemble(res.results, bo2)

